# revision 22
# baseline (speedup 1.0000x reference)
"""Mixtral sparse MoE block on 8 Trainium2 NeuronCores.

Strategy: expert-parallel. Each of the 8 cores owns one expert's weights
(w1[e], w2[e], w3[e]). The host routes tokens: for each expert, gather the
tokens that selected it (<= capacity C, padded with zeros), ship the
gathered tokens transposed plus the expert's weights, and the core
computes yT = (silu(xg @ w1) * (xg @ w2)) @ w3 transposed, densely over
the C token slots. The host applies the routing gates and scatter-adds
each expert's rows back into the full [T, H] output.

Matmuls run in float32r (fp32 with mantissa truncated to 11 bits, full PE
rate) with fp32 PSUM accumulation. Operands are pre-rounded to fp32r on
the host (round-to-nearest-even on the low 12 bits), which the BIR
verifier requires of fp32r matmul producers; the PE then reproduces the
rounded-f64 product exactly (~2.5e-4 relative error overall).

All DRAM tensors are pre-tiled on the host into per-partition-contiguous
layouts so every DMA descriptor moves >= 8KB contiguously (the naive
row-major layouts produce 512B runs and cap DMA at ~65% of line rate).
"""

import sys

for _p in ("/opt/trn_rl_repo", "/root/.axon_site/_ro/trn_rl_repo"):
    if _p not in sys.path:
        sys.path.append(_p)

import numpy as np

H = 2048  # hidden dim
F = 4096  # ffn dim
E = 8     # experts
C = 512   # per-expert token capacity per dispatch round
HT = H // 128
FT = F // 128

_COMPILED = {}

# set by a driver (e.g. test.py) to profile the next dispatch
TRACE = False
LAST_EXEC_NS = None
LAST_RESULTS = None


def _ensure_ntff_hook():
    """Install antenv.axon_hooks shim + register the axon NTFF profile hook
    if the image's antenv package lacks it. Only needed for TRACE runs."""
    try:
        from antenv import axon_hooks  # noqa: F401
        return
    except ImportError:
        pass
    import types
    import antenv

    mod = types.ModuleType("antenv.axon_hooks")
    _hook = [None]
    mod.set_axon_ntff_profile_hook = lambda h: _hook.__setitem__(0, h)
    mod.get_axon_ntff_profile_hook = lambda: _hook[0]
    sys.modules["antenv.axon_hooks"] = mod
    antenv.axon_hooks = mod
    try:
        from trn_agent_boot.trn_boot import _ntff_profile_via_ctypes
        mod.set_axon_ntff_profile_hook(
            _ntff_profile_via_ctypes("/opt/axon/libaxon_pjrt.so")
        )
    except Exception:
        pass


def _round_fp32r(x: np.ndarray) -> np.ndarray:
    """Round fp32 to fp32r (1s+8e+11m, low 12 mantissa bits zeroed, RNE)."""
    b = np.ascontiguousarray(x, dtype=np.float32).view(np.uint32).astype(np.uint64)
    b = b + 0x7FF + ((b >> 12) & 1)
    b = (b & 0xFFFFF000).astype(np.uint32)
    return b.view(np.float32)


def _strip_mm_incs(nc):
    """Remove semaphore increments from matmuls whose tick no instruction
    waits on, renumbering the remaining waits. Every MM carries a +1 inc
    for Tile's vector clock, but only group-final MMs have cross-engine
    consumers; the other ~1400 incs serialize ~15ns each on the PE EVT_SEM
    port. Bails out (no-op) on anything unexpected."""
    fn = nc.m.functions[0]
    insts = []
    for blk in fn.blocks:
        insts.extend(blk.instructions)

    # collect per-sem-id: ordered inc carriers, wait references, and flags
    inc_seq = {}     # sem id -> list of (inst, update_obj)
    waits = {}       # sem id -> list of wait objects (ge-imm)
    unsafe = set()   # sem ids we must not touch
    for inst in insts:
        si = inst.sync_info
        if si is None:
            continue
        for u in si.on_update:
            if u.sync_type != "semaphore":
                continue
            if u.update_mode == "sem-inc" and u.update_value == 1:
                inc_seq.setdefault(u.id, []).append((inst, u))
            else:
                unsafe.add(u.id)
        for w in si.on_wait:
            if w.sync_type != "semaphore":
                continue
            if w.wait_mode == "sem-ge-imm" and w.wait_reg is None:
                waits.setdefault(w.id, []).append(w)
            else:
                unsafe.add(w.id)

    for sem_id, carriers in inc_seq.items():
        if sem_id in unsafe:
            continue
        wlist = waits.get(sem_id, [])
        if any(w.wait_value > len(carriers) or w.wait_value < 1 for w in wlist):
            continue
        referenced = set(w.wait_value for w in wlist)  # 1-based tick values
        # only strip matmul-carried incs; keep everything else
        keep = []
        for tick, (inst, u) in enumerate(carriers, start=1):
            if type(inst).__name__ == "InstMatmult" and tick not in referenced:
                keep.append(False)
            else:
                keep.append(True)
        if all(keep):
            continue
        # old tick -> new tick (rank among kept)
        new_tick = {}
        rank = 0
        for tick, k in enumerate(keep, start=1):
            if k:
                rank += 1
            new_tick[tick] = rank
        for inst, u in (c for c, k in zip(carriers, keep) if not k):
            si = inst.sync_info
            si.on_update = [
                x for x in si.on_update
                if not (x.sync_type == "semaphore" and x.id == sem_id)
            ]
        for w in wlist:
            w.wait_value = new_tick[w.wait_value]


def _build(cap: int):
    import concourse.bacc as bacc
    import concourse.tile as tile
    from concourse import mybir

    F32 = mybir.dt.float32
    F32R = mybir.dt.float32r

    nc = bacc.Bacc("TRN2", target_bir_lowering=False, debug=False, num_devices=E)
    # pre-tiled layouts (see kernel() for the host-side packing):
    #   xgT[p, t, c]     = x_gathered[c, t*128+p]
    #   w1[p, fc, t, j]  = w1_e[t*128+p, fc*128+j]   (w2 identical)
    #   w3[p, ht, fc, j] = w3_e[fc*128+p, ht*128+j]
    #   yT[p, t, c]      = y[c, t*128+p]
    xgT = nc.dram_tensor("xgT", [128, HT, cap], F32R, kind="ExternalInput").ap()
    w1 = nc.dram_tensor("w1", [128, FT, HT, 128], F32R, kind="ExternalInput").ap()
    w2 = nc.dram_tensor("w2", [128, FT, HT, 128], F32R, kind="ExternalInput").ap()
    w3 = nc.dram_tensor("w3", [128, HT, FT, 128], F32R, kind="ExternalInput").ap()
    yT = nc.dram_tensor("yT", [128, HT, cap], F32, kind="ExternalOutput").ap()

    with tile.TileContext(nc) as tc:
        with (
            tc.tile_pool(name="resident", bufs=1) as resident,
            tc.tile_pool(name="wpool", bufs=3) as wpool,
            tc.tile_pool(name="w3pool", bufs=2) as w3pool,
            tc.tile_pool(name="spool", bufs=2) as spool,
            tc.tile_pool(name="ypool", bufs=2) as ypool,
            tc.tile_pool(name="psAB", bufs=3, space="PSUM") as psAB,
            tc.tile_pool(name="psY", bufs=2, space="PSUM") as psY,
        ):
            xg_s = resident.tile([128, HT, cap], F32R)
            hT_s = resident.tile([128, FT, cap], F32R)

            # HAM warmup: dummy matmuls on a const tile run while the
            # first token/weight DMAs stream in, so the PE clock-gate is
            # already released (2.4GHz) when real matmuls start.
            warm = resident.tile([128, 512], mybir.dt.bfloat16)
            nc.gpsimd.memset(warm[:], 0.0)
            for i in range(24):
                pw = psY.tile([128, cap], F32, tag="py")
                nc.tensor.matmul(
                    pw[:], warm[:, :128], warm[:, :cap], start=True, stop=True
                )

            # Phase A: hT[f] = silu(w1_f.T @ xg) * (w2_f.T @ xg)  (F on parts)
            for f in range(FT):
                w1c = wpool.tile([128, HT, 128], F32R, tag="w1c")
                w2c = wpool.tile([128, HT, 128], F32R, tag="w2c")
                if f == 0:
                    # ramp: first tokens-half, then w1c[0] (enables the
                    # first 8 matmuls), then the rest, then w2c[0]
                    h2 = HT // 2
                    nc.sync.dma_start(xg_s[:, :h2, :], xgT[:, :h2, :])
                    nc.sync.dma_start(w1c[:, :h2], w1[:, f, :h2])
                    nc.sync.dma_start(xg_s[:, h2:, :], xgT[:, h2:, :])
                    nc.sync.dma_start(w1c[:, h2:], w1[:, f, h2:])
                else:
                    nc.sync.dma_start(w1c[:], w1[:, f])
                nc.sync.dma_start(w2c[:], w2[:, f])

                pa = psAB.tile([128, cap], F32, tag="pa")
                for t in range(HT):
                    nc.tensor.matmul(
                        pa[:], w1c[:, t, :], xg_s[:, t, :],
                        start=(t == 0), stop=(t == HT - 1),
                    )
                pb = psAB.tile([128, cap], F32, tag="pb")
                for t in range(HT):
                    nc.tensor.matmul(
                        pb[:], w2c[:, t, :], xg_s[:, t, :],
                        start=(t == 0), stop=(t == HT - 1),
                    )
                sa = spool.tile([128, cap], F32, tag="sa")
                nc.scalar.activation(sa[:], pa[:], mybir.ActivationFunctionType.Silu)
                nc.vector.tensor_mul(hT_s[:, f, :], sa[:], pb[:])

            # Phase B: yT[ht] = sum_f w3_chunk(ht,f).T @ hT[f]  (H on parts)
            for t in range(HT):
                w3c = w3pool.tile([128, FT, 128], F32R, tag="w3c")
                nc.sync.dma_start(w3c[:, :FT // 2], w3[:, t, :FT // 2])
                nc.sync.dma_start(w3c[:, FT // 2:], w3[:, t, FT // 2:])
                py = psY.tile([128, cap], F32, tag="py")
                for f in range(FT):
                    nc.tensor.matmul(
                        py[:], w3c[:, f, :], hT_s[:, f, :],
                        start=(f == 0), stop=(f == FT - 1),
                    )
                yt = ypool.tile([128, cap], F32, tag="yt")
                nc.vector.tensor_copy(yt[:], py[:])
                nc.sync.dma_start(yT[:, t, :], yt[:])

    nc.compile()
    _strip_mm_incs(nc)
    return nc


def _get_compiled(cap: int):
    if cap not in _COMPILED:
        _COMPILED[cap] = _build(cap)
    return _COMPILED[cap]


def kernel(hidden_states, selected_experts, routing_weights, w1, w2, w3):
    global LAST_EXEC_NS, LAST_RESULTS
    from concourse.bass_utils import run_bass_kernel_spmd

    hs = np.ascontiguousarray(np.asarray(hidden_states), dtype=np.float32)
    sel = np.asarray(selected_experts)
    rw = np.ascontiguousarray(np.asarray(routing_weights), dtype=np.float32)
    w1 = np.asarray(w1)
    w2 = np.asarray(w2)
    w3 = np.asarray(w3)

    T = hs.shape[0]
    K = sel.shape[1]
    assert hs.shape[1] == H and w1.shape == (E, H, F) and w3.shape == (E, F, H)

    # host routing: gate[t, e] = sum_k rw[t, k] * (sel[t, k] == e)
    gate = np.zeros((T, E), np.float32)
    member = np.zeros((T, E), bool)
    tix = np.arange(T)
    for k in range(K):
        np.add.at(gate, (tix, sel[:, k]), rw[:, k])
        member[tix, sel[:, k]] = True
    idx = [np.nonzero(member[:, e])[0] for e in range(E)]

    xr = _round_fp32r(hs)  # [T, H]
    # pre-tile weights into per-partition-contiguous layouts (f32r-rounded)
    w1p = [
        np.ascontiguousarray(
            _round_fp32r(w1[e]).reshape(HT, 128, FT, 128).transpose(1, 2, 0, 3)
        )
        for e in range(E)
    ]
    w2p = [
        np.ascontiguousarray(
            _round_fp32r(w2[e]).reshape(HT, 128, FT, 128).transpose(1, 2, 0, 3)
        )
        for e in range(E)
    ]
    w3p = [
        np.ascontiguousarray(
            _round_fp32r(w3[e]).reshape(FT, 128, HT, 128).transpose(1, 2, 0, 3)
        )
        for e in range(E)
    ]

    if TRACE:
        _ensure_ntff_hook()
    maxc = max(len(i) for i in idx)
    cap = min(C, max(256, ((maxc + 7) // 8) * 8))
    nc = _get_compiled(cap)
    out = np.zeros((T, H), np.float32)
    rounds = max(1, -(-maxc // cap))
    for r in range(rounds):
        in_maps = []
        for e in range(E):
            ii = idx[e][r * cap:(r + 1) * cap]
            xgT = np.zeros((128, HT, cap), np.float32)
            if len(ii):
                # xgT[p, t, :n] = xr[ii][:, t*128+p].T
                xgT[:, :, :len(ii)] = (
                    xr[ii].reshape(len(ii), HT, 128).transpose(2, 1, 0)
                )
            in_maps.append({
                "xgT": xgT,
                "w1": w1p[e],
                "w2": w2p[e],
                "w3": w3p[e],
            })
        res = run_bass_kernel_spmd(
            nc, in_maps, core_ids=list(range(E)),
            trace=TRACE, trace_cores=(list(range(E)) if TRACE else None),
        )
        if TRACE:
            LAST_EXEC_NS = res.exec_time_ns
            LAST_RESULTS = res
        for e in range(E):
            ii = idx[e][r * cap:(r + 1) * cap]
            if not len(ii):
                continue
            yT = res.results[e]["yT"]  # [128, HT, cap] f32
            y = yT.transpose(2, 1, 0).reshape(cap, H)  # [cap, H]
            out[ii] += gate[ii, e:e + 1] * y[:len(ii)]
    return out


# revision 24
# speedup vs baseline: 1.0061x; 1.0061x over previous
"""Mixtral sparse MoE block on 8 Trainium2 NeuronCores.

Strategy: expert-parallel. Each of the 8 cores owns one expert's weights
(w1[e], w2[e], w3[e]). The host routes tokens: for each expert, gather the
tokens that selected it (<= capacity C, padded with zeros), ship the
gathered tokens transposed plus the expert's weights, and the core
computes yT = (silu(xg @ w1) * (xg @ w2)) @ w3 transposed, densely over
the C token slots. The host applies the routing gates and scatter-adds
each expert's rows back into the full [T, H] output.

Matmuls run in float32r (fp32 with mantissa truncated to 11 bits, full PE
rate) with fp32 PSUM accumulation. Operands are pre-rounded to fp32r on
the host (round-to-nearest-even on the low 12 bits), which the BIR
verifier requires of fp32r matmul producers; the PE then reproduces the
rounded-f64 product exactly (~2.5e-4 relative error overall).

All DRAM tensors are pre-tiled on the host into per-partition-contiguous
layouts so every DMA descriptor moves >= 8KB contiguously (the naive
row-major layouts produce 512B runs and cap DMA at ~65% of line rate).
"""

import sys

for _p in ("/opt/trn_rl_repo", "/root/.axon_site/_ro/trn_rl_repo"):
    if _p not in sys.path:
        sys.path.append(_p)

import numpy as np

H = 2048  # hidden dim
F = 4096  # ffn dim
E = 8     # experts
C = 512   # per-expert token capacity per dispatch round
HT = H // 128
FT = F // 128

_COMPILED = {}

# set by a driver (e.g. test.py) to profile the next dispatch
TRACE = False
LAST_EXEC_NS = None
LAST_RESULTS = None


def _ensure_ntff_hook():
    """Install antenv.axon_hooks shim + register the axon NTFF profile hook
    if the image's antenv package lacks it. Only needed for TRACE runs."""
    try:
        from antenv import axon_hooks  # noqa: F401
        return
    except ImportError:
        pass
    import types
    import antenv

    mod = types.ModuleType("antenv.axon_hooks")
    _hook = [None]
    mod.set_axon_ntff_profile_hook = lambda h: _hook.__setitem__(0, h)
    mod.get_axon_ntff_profile_hook = lambda: _hook[0]
    sys.modules["antenv.axon_hooks"] = mod
    antenv.axon_hooks = mod
    try:
        from trn_agent_boot.trn_boot import _ntff_profile_via_ctypes
        mod.set_axon_ntff_profile_hook(
            _ntff_profile_via_ctypes("/opt/axon/libaxon_pjrt.so")
        )
    except Exception:
        pass


def _round_fp32r(x: np.ndarray) -> np.ndarray:
    """Round fp32 to fp32r (1s+8e+11m, low 12 mantissa bits zeroed, RNE)."""
    b = np.ascontiguousarray(x, dtype=np.float32).view(np.uint32).astype(np.uint64)
    b = b + 0x7FF + ((b >> 12) & 1)
    b = (b & 0xFFFFF000).astype(np.uint32)
    return b.view(np.float32)


def _build(cap: int):
    import concourse.bacc as bacc
    import concourse.tile as tile
    from concourse import mybir

    F32 = mybir.dt.float32
    F32R = mybir.dt.float32r

    nc = bacc.Bacc("TRN2", target_bir_lowering=False, debug=False, num_devices=E)
    # pre-tiled layouts (see kernel() for the host-side packing):
    #   xgT[p, t, c]     = x_gathered[c, t*128+p]
    #   w1[p, fc, t, j]  = w1_e[t*128+p, fc*128+j]   (w2 identical)
    #   w3[p, ht, fc, j] = w3_e[fc*128+p, ht*128+j]
    #   yT[p, t, c]      = y[c, t*128+p]
    xgT = nc.dram_tensor("xgT", [128, HT, cap], F32R, kind="ExternalInput").ap()
    w1 = nc.dram_tensor("w1", [128, FT, HT, 128], F32R, kind="ExternalInput").ap()
    w2 = nc.dram_tensor("w2", [128, FT, HT, 128], F32R, kind="ExternalInput").ap()
    w3 = nc.dram_tensor("w3", [128, HT, FT, 128], F32R, kind="ExternalInput").ap()
    yT = nc.dram_tensor("yT", [128, HT, cap], F32, kind="ExternalOutput").ap()

    with tile.TileContext(nc) as tc:
        with (
            tc.tile_pool(name="resident", bufs=1) as resident,
            tc.tile_pool(name="wpool", bufs=3) as wpool,
            tc.tile_pool(name="w3pool", bufs=2) as w3pool,
            tc.tile_pool(name="spool", bufs=2) as spool,
            tc.tile_pool(name="ypool", bufs=2) as ypool,
            tc.tile_pool(name="psAB", bufs=3, space="PSUM") as psAB,
            tc.tile_pool(name="psY", bufs=2, space="PSUM") as psY,
        ):
            xg_s = resident.tile([128, HT, cap], F32R)
            hT_s = resident.tile([128, FT, cap], F32R)

            # HAM warmup: dummy matmuls on a const tile run while the
            # first token/weight DMAs stream in, so the PE clock-gate is
            # already released (2.4GHz) when real matmuls start.
            warm = resident.tile([128, 512], mybir.dt.bfloat16)
            nc.gpsimd.memset(warm[:], 0.0)
            for i in range(24):
                pw = psY.tile([128, cap], F32, tag="py")
                nc.tensor.matmul(
                    pw[:], warm[:, :128], warm[:, :cap], start=True, stop=True
                )

            # Phase A: hT[f] = silu(w1_f.T @ xg) * (w2_f.T @ xg)  (F on parts)
            for f in range(FT):
                w1c = wpool.tile([128, HT, 128], F32R, tag="w1c")
                w2c = wpool.tile([128, HT, 128], F32R, tag="w2c")
                if f == 0:
                    # ramp: first tokens-half, then w1c[0] (enables the
                    # first 8 matmuls), then the rest, then w2c[0]
                    h2 = HT // 2
                    nc.sync.dma_start(xg_s[:, :h2, :], xgT[:, :h2, :])
                    nc.sync.dma_start(w1c[:, :h2], w1[:, f, :h2])
                    nc.sync.dma_start(xg_s[:, h2:, :], xgT[:, h2:, :])
                    nc.sync.dma_start(w1c[:, h2:], w1[:, f, h2:])
                else:
                    nc.sync.dma_start(w1c[:], w1[:, f])
                nc.sync.dma_start(w2c[:], w2[:, f])

                pa = psAB.tile([128, cap], F32, tag="pa")
                for t in range(HT):
                    nc.tensor.matmul(
                        pa[:], w1c[:, t, :], xg_s[:, t, :],
                        start=(t == 0), stop=(t == HT - 1),
                    )
                pb = psAB.tile([128, cap], F32, tag="pb")
                for t in range(HT):
                    nc.tensor.matmul(
                        pb[:], w2c[:, t, :], xg_s[:, t, :],
                        start=(t == 0), stop=(t == HT - 1),
                    )
                sa = spool.tile([128, cap], F32, tag="sa")
                nc.scalar.activation(sa[:], pa[:], mybir.ActivationFunctionType.Silu)
                nc.vector.tensor_mul(hT_s[:, f, :], sa[:], pb[:])

            # Phase B: yT[ht] = sum_f w3_chunk(ht,f).T @ hT[f]  (H on parts)
            for t in range(HT):
                w3c = w3pool.tile([128, FT, 128], F32R, tag="w3c")
                nc.sync.dma_start(w3c[:, :FT // 2], w3[:, t, :FT // 2])
                nc.sync.dma_start(w3c[:, FT // 2:], w3[:, t, FT // 2:])
                py = psY.tile([128, cap], F32, tag="py")
                for f in range(FT):
                    nc.tensor.matmul(
                        py[:], w3c[:, f, :], hT_s[:, f, :],
                        start=(f == 0), stop=(f == FT - 1),
                    )
                yt = ypool.tile([128, cap], F32, tag="yt")
                nc.vector.tensor_copy(yt[:], py[:])
                nc.sync.dma_start(yT[:, t, :], yt[:])

    nc.compile()
    return nc


def _get_compiled(cap: int):
    if cap not in _COMPILED:
        _COMPILED[cap] = _build(cap)
    return _COMPILED[cap]


def kernel(hidden_states, selected_experts, routing_weights, w1, w2, w3):
    global LAST_EXEC_NS, LAST_RESULTS
    from concourse.bass_utils import run_bass_kernel_spmd

    hs = np.ascontiguousarray(np.asarray(hidden_states), dtype=np.float32)
    sel = np.asarray(selected_experts)
    rw = np.ascontiguousarray(np.asarray(routing_weights), dtype=np.float32)
    w1 = np.asarray(w1)
    w2 = np.asarray(w2)
    w3 = np.asarray(w3)

    T = hs.shape[0]
    K = sel.shape[1]
    assert hs.shape[1] == H and w1.shape == (E, H, F) and w3.shape == (E, F, H)

    # host routing: gate[t, e] = sum_k rw[t, k] * (sel[t, k] == e)
    gate = np.zeros((T, E), np.float32)
    member = np.zeros((T, E), bool)
    tix = np.arange(T)
    for k in range(K):
        np.add.at(gate, (tix, sel[:, k]), rw[:, k])
        member[tix, sel[:, k]] = True
    idx = [np.nonzero(member[:, e])[0] for e in range(E)]

    xr = _round_fp32r(hs)  # [T, H]
    # pre-tile weights into per-partition-contiguous layouts (f32r-rounded)
    w1p = [
        np.ascontiguousarray(
            _round_fp32r(w1[e]).reshape(HT, 128, FT, 128).transpose(1, 2, 0, 3)
        )
        for e in range(E)
    ]
    w2p = [
        np.ascontiguousarray(
            _round_fp32r(w2[e]).reshape(HT, 128, FT, 128).transpose(1, 2, 0, 3)
        )
        for e in range(E)
    ]
    w3p = [
        np.ascontiguousarray(
            _round_fp32r(w3[e]).reshape(FT, 128, HT, 128).transpose(1, 2, 0, 3)
        )
        for e in range(E)
    ]

    if TRACE:
        _ensure_ntff_hook()
    maxc = max(len(i) for i in idx)
    cap = min(C, max(256, ((maxc + 7) // 8) * 8))
    nc = _get_compiled(cap)
    out = np.zeros((T, H), np.float32)
    rounds = max(1, -(-maxc // cap))
    for r in range(rounds):
        in_maps = []
        for e in range(E):
            ii = idx[e][r * cap:(r + 1) * cap]
            xgT = np.zeros((128, HT, cap), np.float32)
            if len(ii):
                # xgT[p, t, :n] = xr[ii][:, t*128+p].T
                xgT[:, :, :len(ii)] = (
                    xr[ii].reshape(len(ii), HT, 128).transpose(2, 1, 0)
                )
            in_maps.append({
                "xgT": xgT,
                "w1": w1p[e],
                "w2": w2p[e],
                "w3": w3p[e],
            })
        res = run_bass_kernel_spmd(
            nc, in_maps, core_ids=list(range(E)),
            trace=TRACE, trace_cores=(list(range(E)) if TRACE else None),
        )
        if TRACE:
            LAST_EXEC_NS = res.exec_time_ns
            LAST_RESULTS = res
        for e in range(E):
            ii = idx[e][r * cap:(r + 1) * cap]
            if not len(ii):
                continue
            yT = res.results[e]["yT"]  # [128, HT, cap] f32
            y = yT.transpose(2, 1, 0).reshape(cap, H)  # [cap, H]
            out[ii] += gate[ii, e:e + 1] * y[:len(ii)]
    return out
